# revision 1
# baseline (speedup 1.0000x reference)
"""Trainium2 Bass kernel for NeuralDisCoCirc: per-sample chain of L=64 GEMVs.

  out_b = (((x_b @ W[b,0] + b[b,0]) @ W[b,1] + b[b,1]) ... @ W[b,63] + b[b,63])

Sharding: data-parallel over batch B=32 -> 8 cores x 4 samples. No collectives.

Device mapping (per core, S=4 samples, D=512, P=128, C=D/P=4):
  - x kept partition-major: x_tile[p, c] = x[c*128 + p]  (shape [128, 4])
  - weight slab W[s,l] streamed as [128, C, 512]: wt[p, ic, j] = W[ic*128+p, j]
    (a pure reshape of the natural row-major [512, 512] slab -> contiguous DMA)
  - GEMV via 16 accumulating matmuls: for output chunk jc, sum over ic of
      lhsT = wt[:, ic, jc*128:(jc+1)*128]   ([K=128, M=128], stationary)
      rhs  = x_tile[:, ic:ic+1]             ([K=128, N=1], moving)
    -> psum[:, jc] ([128, 1]).  Result is partition-major [128, 4] = exactly
    the layout the next layer's x needs: no transpose anywhere in the chain.
  - bias pre-transposed on host to the same partition-major layout, loaded once.

The 2 GiB of weights is touched exactly once -> DMA-bound; weight DMAs are
1 MiB contiguous slabs, multi-buffered so the tensor engine never waits.
"""

import sys

for _p in ("/opt/trn_rl_repo",):
    if _p not in sys.path:
        sys.path.insert(0, _p)

import numpy as np

B, L, D = 32, 64, 512
NCORES = 8
S = B // NCORES          # samples per core
P = 128                  # SBUF partitions
C = D // P               # 512 = 4 chunks of 128
F32 = None               # set after mybir import

WBUFS = 4                # weight slabs buffered per sample tag

_cached = {}


def _build_program():
    import concourse.bacc as bacc
    import concourse.bass as bass
    import concourse.mybir as mybir
    import concourse.tile as tile

    f32 = mybir.dt.float32
    nc = bacc.Bacc("TRN2", target_bir_lowering=False, debug=False,
                   num_devices=NCORES)

    w_dram = nc.dram_tensor("w", (S, L, D, D), f32, kind="ExternalInput")
    x_dram = nc.dram_tensor("x0", (P, S * C), f32, kind="ExternalInput")
    b_dram = nc.dram_tensor("bvec", (P, S * L * C), f32, kind="ExternalInput")
    o_dram = nc.dram_tensor("out", (P, S * C), f32, kind="ExternalOutput")

    with tile.TileContext(nc) as tc:
        with (
            tc.tile_pool(name="const", bufs=1) as cpool,
            tc.tile_pool(name="wpool", bufs=WBUFS) as wpool,
            tc.tile_pool(name="xpool", bufs=3) as xpool,
            tc.tile_pool(name="psum", bufs=2, space=bass.MemorySpace.PSUM) as ppool,
        ):
            bias = cpool.tile([P, S * L * C], f32, tag="bias")
            nc.sync.dma_start(out=bias[:], in_=b_dram[:])

            xs = []
            for s in range(S):
                xt = xpool.tile([P, C], f32, tag=f"x{s}")
                nc.sync.dma_start(out=xt[:], in_=x_dram[:, s * C:(s + 1) * C])
                xs.append(xt)

            for l in range(L):
                for s in range(S):
                    wt = wpool.tile([P, C, D], f32, tag=f"w{s}")
                    nc.sync.dma_start(
                        out=wt[:],
                        in_=w_dram[s, l].rearrange("(c p) j -> p c j", p=P),
                    )
                    ps = ppool.tile([P, C], f32, tag=f"ps{s}")
                    for jc in range(C):
                        for ic in range(C):
                            nc.tensor.matmul(
                                ps[:, jc:jc + 1],
                                wt[:, ic, jc * P:(jc + 1) * P],
                                xs[s][:, ic:ic + 1],
                                start=(ic == 0),
                                stop=(ic == C - 1),
                            )
                    nxt = xpool.tile([P, C], f32, tag=f"x{s}")
                    off = (s * L + l) * C
                    nc.vector.tensor_add(nxt[:], ps[:], bias[:, off:off + C])
                    xs[s] = nxt

            outt = cpool.tile([P, S * C], f32, tag="outt")
            for s in range(S):
                nc.vector.tensor_copy(outt[:, s * C:(s + 1) * C], xs[s][:])
            nc.sync.dma_start(out=o_dram[:], in_=outt[:])

    nc.compile()
    return nc


def _get_program():
    if "nc" not in _cached:
        _cached["nc"] = _build_program()
    return _cached["nc"]


def kernel(x: np.ndarray, weights: np.ndarray, biases: np.ndarray) -> np.ndarray:
    from concourse.bass_utils import run_bass_kernel_spmd

    x = np.ascontiguousarray(x, dtype=np.float32)
    weights = np.asarray(weights, dtype=np.float32)
    biases = np.asarray(biases, dtype=np.float32)

    nc = _get_program()

    in_maps = []
    for core in range(NCORES):
        lo, hi = core * S, (core + 1) * S
        # x slice [S, D] -> partition-major [P, S*C]: xt[p, s*C+c] = x[s, c*128+p]
        xc = np.ascontiguousarray(
            x[lo:hi].reshape(S, C, P).transpose(2, 0, 1).reshape(P, S * C)
        )
        # biases [S, L, D] -> [P, S*L*C]: bt[p, (s*L+l)*C+c] = biases[s,l,c*128+p]
        bc = np.ascontiguousarray(
            biases[lo:hi].reshape(S, L, C, P).transpose(3, 0, 1, 2).reshape(P, S * L * C)
        )
        wc = weights[lo:hi]  # contiguous view, no copy
        in_maps.append({"w": wc, "x0": xc, "bvec": bc})

    res = run_bass_kernel_spmd(nc, in_maps, core_ids=list(range(NCORES)))
    _cached["last_results"] = res

    out = np.empty((B, D), dtype=np.float32)
    for core in range(NCORES):
        oc = res.results[core]["out"]  # [P, S*C]
        # invert: oc[p, s*C+c] = y[s, c*128+p]
        y = oc.reshape(P, S, C).transpose(1, 2, 0).reshape(S, D)
        out[core * S:(core + 1) * S] = y
    return out
